# revision 15
# baseline (speedup 1.0000x reference)
"""Trainium2 Bass kernel for nn_MCFL_49254684950998 (dense multimodal transformer block).

Strategy: pure data parallel over 8 NeuronCores (batch 16384 -> 2048/core).
All on-device activations are feature-major ("T layout": [feat_chunk=128, batch]);
the host pre-transposes inputs (and converts to bf16) and post-transposes outputs,
so the device does zero transposes.  All GEMMs run as bf16 matmuls (full PE rate,
FWL weight loads) with f32 PSUM accumulation.  Attention (3-token self-attn +
2-token cross-attn) uses DVE elementwise products + tiny selection matmuls on the
TensorEngine (head-segment reductions and probability broadcasts), softmax on
ACT/DVE (reciprocal_approx_fast for denominators), LayerNorm stats via ones-matmul
column sums on PE + gpsimd partition_broadcast.  Weight slabs are streamed per
block (double/triple-buffered); activations for the next block are double-buffered
so the next block's GEMMs fill the softmax/LN bubbles and the PE stays HAM-warm.
"""

import os
import sys

sys.path.insert(0, "/opt/trn_rl_repo")

import numpy as np
import ml_dtypes

import concourse.bass as bass
import concourse.bacc as bacc
import concourse.tile as tile
import concourse.mybir as mybir
from concourse import bass_utils

F32 = mybir.dt.float32
BF16 = mybir.dt.bfloat16
AF = mybir.ActivationFunctionType
OP = mybir.AluOpType

B, D, H, HD = 16384, 1024, 16, 64
NCORES = 8
BLOC = B // NCORES          # 2048 batch rows per core
BF = 512                    # batch tile (free dim) per block
NBLK_HW = BLOC // BF        # 4 blocks per core
NCH = D // 128              # 8 feature chunks
SCALE = HD ** -0.5
EPS = 1e-5


def build(tc, outs, ins, nblk):
    from contextlib import ExitStack
    stack = ExitStack()
    nc = tc.nc
    out_t = outs["out_t"]
    xt = [ins["xt_t"], ins["xt_i"], ins["xt_a"]]

    # ---- const tiles (loaded once) ----
    cpool = stack.enter_context(tc.tile_pool(name="consts", bufs=1))
    sel_sb = cpool.tile([128, NCH * 16], BF16, tag="sel")       # [128, c, 16]
    nc.sync.dma_start(sel_sb[:], ins["sel"][:])
    selb_sb = cpool.tile([96, NCH * 128], BF16, tag="selb")     # selb at bases 0/32/64
    nc.sync.dma_start(selb_sb[:], ins["selb"][:])
    ones_sb = cpool.tile([128, 2], BF16, tag="ones")            # [-1/D, +1/D]
    nc.sync.dma_start(ones_sb[:], ins["ones2"][:])
    id_sb = cpool.tile([128, 128], BF16, tag="ident")
    nc.sync.dma_start(id_sb[:], ins["ident"][:])
    cols = {}
    for nm in ("sab", "l1g", "l1b", "cab", "l2g", "l2b"):
        cols[nm] = cpool.tile([128, NCH], F32, tag=nm, name=f"col_{nm}")
        nc.sync.dma_start(cols[nm][:], ins[nm][:])

    def sel_c(c):
        return sel_sb[:, c * 16:(c + 1) * 16]

    def selb_c(c, base=0):
        return selb_sb[base:base + 16, c * 128:(c + 1) * 128]

    # ---- static activation arenas (bf16) ----
    v_sb = nc.alloc_sbuf_tensor("v_sb", [128, 2 * NCH * BF], BF16).ap()   # self V modes 0,1
    v2_sb = nc.alloc_sbuf_tensor("v2_sb", [128, NCH * BF], BF16).ap()     # self V mode 2
    u_sb = nc.alloc_sbuf_tensor("u_sb", [128, 3 * NCH * BF], BF16).ap()   # attnout per tok
    cq_sb = nc.alloc_sbuf_tensor("cq_sb", [128, NCH * BF], BF16).ap()     # cross q
    aca_sb = nc.alloc_sbuf_tensor("aca_sb", [128, NCH * BF], BF16).ap()   # cross attnout
    ckv_sb = nc.alloc_sbuf_tensor("ckv_sb", [128, 2 * NCH * BF], BF16).ap()  # ck then cv (2 toks)

    def vs(i):
        return v_sb[:, i * BF:(i + 1) * BF]

    def v2s(c):
        return v2_sb[:, c * BF:(c + 1) * BF]

    def us(i):
        return u_sb[:, i * BF:(i + 1) * BF]

    def cqs(c):
        return cq_sb[:, c * BF:(c + 1) * BF]

    def acas(c):
        return aca_sb[:, c * BF:(c + 1) * BF]

    def ckvs(i):
        return ckv_sb[:, i * BF:(i + 1) * BF]

    # softmax smalls: token i occupies partitions [32i, 32i+16)
    E_sb = nc.alloc_sbuf_tensor("E_sb", [96, 3 * BF], BF16).ap()
    P_sb = nc.alloc_sbuf_tensor("P_sb", [96, 3 * BF], BF16).ap()
    esum_sb = nc.alloc_sbuf_tensor("esum_sb", [96, BF], BF16).ap()
    esumf_sb = nc.alloc_sbuf_tensor("esumf_sb", [96, BF], F32).ap()
    rec_sb = nc.alloc_sbuf_tensor("rec_sb", [96, BF], F32).ap()
    E2_sb = nc.alloc_sbuf_tensor("E2_sb", [16, 2 * BF], BF16).ap()
    P2_sb = nc.alloc_sbuf_tensor("P2_sb", [16, 2 * BF], BF16).ap()
    esum2_sb = nc.alloc_sbuf_tensor("esum2_sb", [16, BF], BF16).ap()
    esum2f_sb = nc.alloc_sbuf_tensor("esum2f_sb", [16, BF], F32).ap()
    rec2_sb = nc.alloc_sbuf_tensor("rec2_sb", [16, BF], F32).ap()

    # ---- pools ----
    def pool(*a, **k):
        return stack.enter_context(tc.tile_pool(*a, **k))

    x_pool = pool(name="x", bufs=2)            # [128, 24*BF] bf16 per block
    wq_pool = pool(name="wqkv", bufs=2)        # [128, 8*256] bf16 (q|k slabs)
    wv_pool = pool(name="wv", bufs=2)          # [128, 8*128] bf16 (v slabs)
    og_pool = pool(name="og", bufs=3)          # [128, 8*256] bf16
    qk_pool = pool(name="qk", bufs=7)          # [128, BF] bf16
    prod_pool = pool(name="prod", bufs=5)      # [128, BF] bf16
    pb_pool = pool(name="pb", bufs=3)          # [128, BF] bf16
    rep_pool = pool(name="rep", bufs=3)        # [128, BF] bf16
    sm_pool = pool(name="smalls", bufs=4)      # [1, BF] f32
    smb_pool = pool(name="smallsb", bufs=2)    # [1, BF] bf16
    ps_S = pool(name="psS", bufs=3, space="PSUM")
    ps_main = pool(name="psgen", bufs=5, space="PSUM")    # shared rotating accumulators
    ps_aux = ps_main

    def layernorm(chunks, g_col, b_col):
        """in-place LN over feature dim (partitions across 8 chunk tiles)"""
        nmu_ps = ps_aux.tile([1, BF], F32, tag="ps", name="nmu", padded_shape=[128, BF])
        m2_ps = ps_aux.tile([1, BF], F32, tag="ps", name="m2", padded_shape=[128, BF])
        for c in range(NCH):
            sq = prod_pool.tile([128, BF], BF16, tag="prod")
            nc.scalar.square(sq[:], chunks[c])
            st, sp = (c == 0), (c == NCH - 1)
            nc.tensor.matmul(nmu_ps[:], ones_sb[:, 0:1], chunks[c], start=st, stop=sp)
            nc.tensor.matmul(m2_ps[:], ones_sb[:, 1:2], sq[:], start=st, stop=sp)
        nmu = sm_pool.tile([1, BF], F32, tag="sm")
        nc.vector.tensor_copy(nmu[:], nmu_ps[:])
        nmu_b = smb_pool.tile([1, BF], BF16, tag="smb")
        nc.vector.tensor_copy(nmu_b[:], nmu[:])
        var = sm_pool.tile([1, BF], F32, tag="sm")
        nc.vector.tensor_tensor(var[:], nmu[:], nmu[:], op=OP.mult)        # mu^2
        nc.vector.tensor_tensor(var[:], m2_ps[:], var[:], op=OP.subtract)  # var
        nc.vector.tensor_scalar_add(var[:], var[:], EPS)
        rstd = sm_pool.tile([1, BF], F32, tag="sm")
        nc.scalar.activation(rstd[:], var[:], AF.Abs_reciprocal_sqrt)
        rstd_b = smb_pool.tile([1, BF], BF16, tag="smb")
        nc.vector.tensor_copy(rstd_b[:], rstd[:])
        nmu_rep = rep_pool.tile([128, BF], BF16, tag="rep")
        nc.gpsimd.partition_broadcast(nmu_rep[:], nmu_b[:])
        rstd_rep = rep_pool.tile([128, BF], BF16, tag="rep")
        nc.gpsimd.partition_broadcast(rstd_rep[:], rstd_b[:])
        for c in range(NCH):
            nc.vector.tensor_tensor(chunks[c], chunks[c], nmu_rep[:], op=OP.add)
            nc.vector.tensor_tensor(chunks[c], chunks[c], rstd_rep[:], op=OP.mult)
            nc.gpsimd.tensor_scalar(chunks[c], chunks[c], g_col[:, c:c + 1],
                                    b_col[:, c:c + 1], op0=OP.mult, op1=OP.add)

    xblks = {}

    def load_xblk(blk):
        if blk in xblks or blk >= nblk:
            return
        t = x_pool.tile([128, 3 * NCH * BF], BF16, tag="x")
        bs_ = blk * BF
        for m in range(3):
            for c in range(NCH):
                nc.sync.dma_start(t[:, (m * NCH + c) * BF:(m * NCH + c + 1) * BF],
                                  xt[m][c * 128:(c + 1) * 128, bs_:bs_ + BF])
        xblks[blk] = t

    def xv(blk, m, c):
        xb = xblks[blk]
        return xb[:, (m * NCH + c) * BF:(m * NCH + c + 1) * BF]

    S_of = {}

    def h1a_unit(blk, c):
        """q/k GEMM + self-attn scores for one out-chunk c of block blk"""
        def fn():
            if c == 0:
                S_of[blk] = [ps_S.tile([128, BF], F32, tag="S", name=f"Sb{j}_{blk}")
                             for j in range(3)]
            S_banks = S_of[blk]
            wslab = wq_pool.tile([128, NCH * 256], BF16, tag="wqkv")
            nc.sync.dma_start(wslab[:], ins["wqk_p"][:, c * NCH * 256:(c + 1) * NCH * 256])
            qts, kts = [], []
            for t in range(2):      # 0 -> q, 1 -> k
                for m in range(3):
                    ps = ps_main.tile([128, BF], F32, tag="ps")
                    for k in range(NCH):
                        wk = wslab[:, k * 256 + t * 128:k * 256 + t * 128 + 128]
                        nc.tensor.matmul(ps[:], wk, xv(blk, m, k), start=(k == 0), stop=(k == NCH - 1))
                    sb = qk_pool.tile([128, BF], BF16, tag="qk")
                    nc.scalar.copy(sb[:], ps[:])
                    (qts if t == 0 else kts).append(sb)
            for i in range(3):
                for j in range(3):
                    pr = prod_pool.tile([128, BF], BF16, tag="prod")
                    nc.vector.tensor_tensor(pr[:], qts[i][:], kts[j][:], op=OP.mult)
                    nc.tensor.matmul(
                        S_banks[j][32 * i:32 * i + 16, :],
                        sel_c(c), pr[:],
                        start=(c == 0), stop=(c == NCH - 1),
                        tile_position=(0, 32 * i),
                        skip_group_check=True,
                    )
        return fn

    def h1b_unit(blk, c):
        """v GEMM for one out-chunk c of block blk (emitted after ws(blk-1) v reads)"""
        def fn():
            vslab = wv_pool.tile([128, NCH * 128], BF16, tag="wv")
            nc.sync.dma_start(vslab[:], ins["wv_p"][:, c * NCH * 128:(c + 1) * NCH * 128])
            for m in range(3):
                ps = ps_main.tile([128, BF], F32, tag="ps")
                for k in range(NCH):
                    wk = vslab[:, k * 128:(k + 1) * 128]
                    nc.tensor.matmul(ps[:], wk, xv(blk, m, k), start=(k == 0), stop=(k == NCH - 1))
                dst = v2s(c) if m == 2 else vs(m * NCH + c)
                nc.scalar.copy(dst, ps[:])
        return fn

    def h2_units(blk):
        units = []
        S_banks = None

        def u_softmax():
            S_banks = S_of.pop(blk)
            for j in range(3):
                nc.scalar.activation(E_sb[0:80, j * BF:(j + 1) * BF], S_banks[j][0:80, :], AF.Exp)
            nc.vector.tensor_tensor(esum_sb[0:80, :], E_sb[0:80, 0:BF], E_sb[0:80, BF:2 * BF], op=OP.add)
            nc.vector.tensor_tensor(esum_sb[0:80, :], esum_sb[0:80, :], E_sb[0:80, 2 * BF:3 * BF], op=OP.add)
            nc.vector.tensor_copy(esumf_sb[0:80, :], esum_sb[0:80, :])
            nc.vector.reciprocal_approx_fast(out=rec_sb[0:80, :], in_=esumf_sb[0:80, :])
            for j in range(2):
                nc.vector.tensor_tensor(P_sb[0:80, j * BF:(j + 1) * BF],
                                        E_sb[0:80, j * BF:(j + 1) * BF],
                                        rec_sb[0:80, :], op=OP.mult)
            # dv0 = v0 - v2 in cq arena; dv1 = v1 - v2 in aca arena
            for c in range(NCH):
                nc.vector.tensor_tensor(cqs(c), vs(0 * NCH + c), v2s(c), op=OP.subtract)
                nc.vector.tensor_tensor(acas(c), vs(1 * NCH + c), v2s(c), op=OP.subtract)
        units.append(u_softmax)

        def mk_ws(t):
            def u_ws():
                for c in range(NCH):
                    tts = []
                    for j in range(2):
                        pb_ps = ps_aux.tile([128, BF], F32, tag="ps")
                        nc.tensor.matmul(pb_ps[:], selb_c(c, 32 * t),
                                         P_sb[32 * t:32 * t + 16, j * BF:(j + 1) * BF],
                                         start=True, stop=True)
                        pbs = pb_pool.tile([128, BF], BF16, tag="pb")
                        nc.scalar.copy(pbs[:], pb_ps[:])
                        tt = prod_pool.tile([128, BF], BF16, tag="prod")
                        dv = cqs(c) if j == 0 else acas(c)
                        nc.vector.tensor_tensor(tt[:], pbs[:], dv, op=OP.mult)
                        tts.append(tt)
                    s = prod_pool.tile([128, BF], BF16, tag="prod")
                    nc.vector.tensor_tensor(s[:], tts[0][:], tts[1][:], op=OP.add)
                    nc.gpsimd.tensor_tensor(us(t * NCH + c), s[:], v2s(c), op=OP.add)
            return u_ws
        for t in range(3):
            units.append(mk_ws(t))

        def mk_saln(t):
            def u_saln():
                for og in range(4):
                    slab = og_pool.tile([128, NCH * 256], BF16, tag="og", name="wsa")
                    nc.sync.dma_start(slab[:], ins["wsa_p"][:, og * NCH * 256:(og + 1) * NCH * 256])
                    for oj in range(2):
                        o = og * 2 + oj
                        ps = ps_main.tile([128, BF], F32, tag="ps")
                        for k in range(NCH):
                            wk = slab[:, k * 256 + oj * 128:k * 256 + oj * 128 + 128]
                            nc.tensor.matmul(ps[:], wk, us(t * NCH + k), start=(k == 0), stop=(k == NCH - 1))
                        nc.vector.scalar_tensor_tensor(
                            xv(blk, t, o), ps[:], cols["sab"][:, o:o + 1], xv(blk, t, o),
                            op0=OP.add, op1=OP.add)
                layernorm([xv(blk, t, c) for c in range(NCH)], cols["l1g"], cols["l1b"])
                if t == 0:
                    # cq = tln_text @ Wq
                    for og in range(4):
                        slab = og_pool.tile([128, NCH * 256], BF16, tag="og", name="wwq")
                        nc.sync.dma_start(slab[:], ins["wq_p"][:, og * NCH * 256:(og + 1) * NCH * 256])
                        for oj in range(2):
                            o = og * 2 + oj
                            ps = ps_main.tile([128, BF], F32, tag="ps")
                            for k in range(NCH):
                                wk = slab[:, k * 256 + oj * 128:k * 256 + oj * 128 + 128]
                                nc.tensor.matmul(ps[:], wk, xv(blk, 0, k), start=(k == 0), stop=(k == NCH - 1))
                            nc.scalar.copy(cqs(o), ps[:])
            return u_saln
        for t in range(3):
            units.append(mk_saln(t))

        Sc_of = {}

        def u_ck():
            # ck for img(tok1), aud(tok2): Wkv og 0..3, then cross scores
            for og in range(4):
                slab = og_pool.tile([128, NCH * 256], BF16, tag="og", name="wkv_k")
                nc.sync.dma_start(slab[:], ins["wkv_p"][:, og * NCH * 256:(og + 1) * NCH * 256])
                for oj in range(2):
                    o = og * 2 + oj
                    for t in (1, 2):
                        ps = ps_main.tile([128, BF], F32, tag="ps")
                        for k in range(NCH):
                            wk = slab[:, k * 256 + oj * 128:k * 256 + oj * 128 + 128]
                            nc.tensor.matmul(ps[:], wk, xv(blk, t, k), start=(k == 0), stop=(k == NCH - 1))
                        nc.scalar.copy(ckvs((t - 1) * NCH + o), ps[:])
            Sc = [ps_aux.tile([128, BF], F32, tag="ps", name=f"Sc{jj}") for jj in range(2)]
            Sc_of[0] = Sc
            for c in range(NCH):
                for jj in range(2):
                    pr = prod_pool.tile([128, BF], BF16, tag="prod")
                    nc.vector.tensor_tensor(pr[:], cqs(c), ckvs(jj * NCH + c), op=OP.mult)
                    nc.tensor.matmul(Sc[jj][0:16, :], sel_c(c), pr[:],
                                     start=(c == 0), stop=(c == NCH - 1))
        units.append(u_ck)

        def u_cv():
            # cv pass (Wkv og 4..7) — independent of cross softmax, fills its latency
            for og in range(4, 8):
                slab = og_pool.tile([128, NCH * 256], BF16, tag="og", name="wkv_v")
                nc.sync.dma_start(slab[:], ins["wkv_p"][:, og * NCH * 256:(og + 1) * NCH * 256])
                for oj in range(2):
                    o = (og - 4) * 2 + oj
                    for t in (1, 2):
                        ps = ps_main.tile([128, BF], F32, tag="ps")
                        for k in range(NCH):
                            wk = slab[:, k * 256 + oj * 128:k * 256 + oj * 128 + 128]
                            nc.tensor.matmul(ps[:], wk, xv(blk, t, k), start=(k == 0), stop=(k == NCH - 1))
                        nc.scalar.copy(ckvs((t - 1) * NCH + o), ps[:])
        units.append(u_cv)

        def u_sm2():
            Sc = Sc_of.pop(0)
            for jj in range(2):
                nc.scalar.activation(E2_sb[0:16, jj * BF:(jj + 1) * BF], Sc[jj][0:16, :], AF.Exp)
            nc.vector.tensor_tensor(esum2_sb[0:16, :], E2_sb[0:16, 0:BF], E2_sb[0:16, BF:2 * BF], op=OP.add)
            nc.vector.tensor_copy(esum2f_sb[0:16, :], esum2_sb[0:16, :])
            nc.vector.reciprocal_approx_fast(out=rec2_sb[0:16, :], in_=esum2f_sb[0:16, :])
            nc.vector.tensor_tensor(P2_sb[0:16, 0:BF], E2_sb[0:16, 0:BF],
                                    rec2_sb[0:16, :], op=OP.mult)
        units.append(u_sm2)

        def u_aca():
            # weighted cv sum -> aca_sb (telescoped: P_aud = 1 - P_img)
            for c in range(NCH):
                pb_ps = ps_aux.tile([128, BF], F32, tag="ps")
                nc.tensor.matmul(pb_ps[:], selb_c(c), P2_sb[0:16, 0:BF], start=True, stop=True)
                pbs = pb_pool.tile([128, BF], BF16, tag="pb")
                nc.scalar.copy(pbs[:], pb_ps[:])
                dvc = prod_pool.tile([128, BF], BF16, tag="prod")
                nc.vector.tensor_tensor(dvc[:], ckvs(0 * NCH + c), ckvs(1 * NCH + c), op=OP.subtract)
                tt = prod_pool.tile([128, BF], BF16, tag="prod")
                nc.vector.tensor_tensor(tt[:], pbs[:], dvc[:], op=OP.mult)
                nc.gpsimd.tensor_tensor(acas(c), tt[:], ckvs(1 * NCH + c), op=OP.add)
        units.append(u_aca)

        def u_ca():
            for og in range(4):
                slab = og_pool.tile([128, NCH * 256], BF16, tag="og", name="wca")
                nc.sync.dma_start(slab[:], ins["wca_p"][:, og * NCH * 256:(og + 1) * NCH * 256])
                for oj in range(2):
                    o = og * 2 + oj
                    ps = ps_main.tile([128, BF], F32, tag="ps")
                    for k in range(NCH):
                        wk = slab[:, k * 256 + oj * 128:k * 256 + oj * 128 + 128]
                        nc.tensor.matmul(ps[:], wk, acas(k), start=(k == 0), stop=(k == NCH - 1))
                    nc.vector.scalar_tensor_tensor(
                        xv(blk, 0, o), ps[:], cols["cab"][:, o:o + 1], xv(blk, 0, o),
                        op0=OP.add, op1=OP.add)
            layernorm([xv(blk, 0, c) for c in range(NCH)], cols["l2g"], cols["l2b"])
        units.append(u_ca)

        def u_store():
            bs = blk * BF
            for c in range(NCH):
                nc.scalar.dma_start(out_t[c * 128:(c + 1) * 128, bs:bs + BF], xv(blk, 0, c))
            xblks.pop(blk)
            load_xblk(blk + 2)
        units.append(u_store)
        return units

    # ---- software-pipelined emission: H1(blk+1) units interleaved into H2(blk) ----
    load_xblk(0)
    for c in range(NCH):
        h1a_unit(0, c)()
    for c in range(NCH):
        h1b_unit(0, c)()
    load_xblk(1)
    for blk in range(nblk):
        B = h2_units(blk)             # [sm, ws0, ws1, ws2, saln0, saln1, saln2, ck, cv, sm2, aca, ca, store]
        nxt = blk + 1 < nblk
        A1 = [h1a_unit(blk + 1, c) for c in range(NCH)] if nxt else []
        A2 = [h1b_unit(blk + 1, c) for c in range(NCH)] if nxt else []
        sm, ws0, ws1, ws2, saln0, saln1, saln2, ck, cv, sm2, aca, ca, store = B
        if nxt:
            order = [sm, A1[0], ws0, A1[1], ws1, A1[2], ws2, A1[3],
                     saln0, A1[4], A2[0], saln1, A1[5], A2[1],
                     saln2, A1[6], A2[2], ck, A1[7], A2[3],
                     cv, A2[4], sm2, A2[5], aca, A2[6], ca, A2[7], store]
        else:
            order = list(B)
        for u in order:
            u()

    stack.close()


# ------------------------------------------------------------------ host side

def _prep_shared(Wqkv, sa_proj_w, sa_proj_b, ln1_g, ln1_b, Wq, Wkv, ca_proj_w,
                 ca_proj_b, ln2_g, ln2_b):
    f = np.float32
    bf = ml_dtypes.bfloat16

    def kperm(W):  # [1024, N] -> [128, og, 8k, 256] flat (og-contiguous slabs)
        N = W.shape[1]
        kp = W.reshape(NCH, 128, N).transpose(1, 0, 2)          # [128, 8k, N]
        nog = N // 256
        og = kp.reshape(128, NCH, nog, 256).transpose(0, 2, 1, 3)  # [128, og, k, 256]
        return np.ascontiguousarray(og.reshape(128, N * NCH)).astype(bf)

    # Wqkv split: per out-chunk c: [q_c | k_c] and [v_c] -> [128, 8c, 8k, 256] / [..., 128]
    Wq3 = np.asarray(Wqkv, f).reshape(1024, 3, NCH, 128)   # [k, qkv, c, 128]
    per_c_qk, per_c_v = [], []
    for c in range(NCH):
        qk = np.concatenate([Wq3[:, 0, c, :], Wq3[:, 1, c, :]], axis=1)  # [1024, 256]
        per_c_qk.append(qk.reshape(NCH, 128, 256).transpose(1, 0, 2).reshape(128, NCH * 256))
        per_c_v.append(Wq3[:, 2, c, :].reshape(NCH, 128, 128).transpose(1, 0, 2).reshape(128, NCH * 128))
    wqk_p = np.ascontiguousarray(np.concatenate(per_c_qk, axis=1)).astype(bf)
    wv_p = np.ascontiguousarray(np.concatenate(per_c_v, axis=1)).astype(bf)

    sel = np.zeros((128, NCH, 16), f)
    for r in range(128):
        for c in range(NCH):
            sel[r, c, 2 * c + r // 64] = SCALE
    selb1 = np.zeros((16, NCH, 128), f)
    for h in range(16):
        for c in range(NCH):
            for m in range(128):
                if h == 2 * c + m // 64:
                    selb1[h, c, m] = 1.0
    selb = np.zeros((96, NCH, 128), f)
    for b0 in (0, 32, 64):
        selb[b0:b0 + 16] = selb1
    col = lambda v: np.ascontiguousarray(np.asarray(v, f).reshape(NCH, 128).T)
    ones2 = np.stack([np.full((128,), -1.0 / D, f), np.full((128,), 1.0 / D, f)], axis=1)
    ident = np.eye(128, dtype=f)
    return {
        "wqk_p": wqk_p,
        "wv_p": wv_p,
        "wsa_p": kperm(np.asarray(sa_proj_w, f)),
        "wq_p": kperm(np.asarray(Wq, f)),
        "wkv_p": kperm(np.asarray(Wkv, f)),
        "wca_p": kperm(np.asarray(ca_proj_w, f)),
        "sel": sel.reshape(128, NCH * 16).astype(bf),
        "selb": selb.reshape(96, NCH * 128).astype(bf),
        "ones2": ones2.astype(bf),
        "ident": ident.astype(bf),
        "sab": col(sa_proj_b), "l1g": col(ln1_g), "l1b": col(ln1_b),
        "cab": col(ca_proj_b), "l2g": col(ln2_g), "l2b": col(ln2_b),
    }


_CACHE = {}


def _get_program(nblk):
    if nblk in _CACHE:
        return _CACHE[nblk]
    nc = bacc.Bacc("TRN2", target_bir_lowering=False, debug=False,
                   enable_asserts=False, num_devices=NCORES)
    ins = {}
    bl = nblk * BF
    for nm in ("xt_t", "xt_i", "xt_a"):
        ins[nm] = nc.dram_tensor(nm, [D, bl], BF16, kind="ExternalInput").ap()
    ins["wqk_p"] = nc.dram_tensor("wqk_p", [128, NCH * NCH * 256], BF16, kind="ExternalInput").ap()
    ins["wv_p"] = nc.dram_tensor("wv_p", [128, NCH * NCH * 128], BF16, kind="ExternalInput").ap()
    for nm, w in (("wsa_p", 1024), ("wq_p", 1024), ("wkv_p", 2048), ("wca_p", 1024)):
        ins[nm] = nc.dram_tensor(nm, [128, NCH * w], BF16, kind="ExternalInput").ap()
    ins["sel"] = nc.dram_tensor("sel", [128, NCH * 16], BF16, kind="ExternalInput").ap()
    ins["selb"] = nc.dram_tensor("selb", [96, NCH * 128], BF16, kind="ExternalInput").ap()
    ins["ones2"] = nc.dram_tensor("ones2", [128, 2], BF16, kind="ExternalInput").ap()
    ins["ident"] = nc.dram_tensor("ident", [128, 128], BF16, kind="ExternalInput").ap()
    for nm in ("sab", "l1g", "l1b", "cab", "l2g", "l2b"):
        ins[nm] = nc.dram_tensor(nm, [128, NCH], F32, kind="ExternalInput").ap()
    outs = {"out_t": nc.dram_tensor("out_t", [D, bl], BF16, kind="ExternalOutput").ap()}

    with tile.TileContext(nc) as tc:
        build(tc, outs, ins, nblk)
    nc.compile()
    _CACHE[nblk] = nc
    return nc


def kernel(c_text, c_image, c_audio, Wqkv, sa_proj_w, sa_proj_b, ln1_g, ln1_b,
           Wq, Wkv, ca_proj_w, ca_proj_b, ln2_g, ln2_b, _trace=False):
    bf = ml_dtypes.bfloat16
    shared = _prep_shared(Wqkv, sa_proj_w, sa_proj_b, ln1_g, ln1_b, Wq, Wkv,
                          ca_proj_w, ca_proj_b, ln2_g, ln2_b)
    in_maps = []
    for s in range(NCORES):
        sl = slice(s * BLOC, (s + 1) * BLOC)
        m = dict(shared)
        m["xt_t"] = np.ascontiguousarray(np.asarray(c_text, np.float32)[sl].T).astype(bf)
        m["xt_i"] = np.ascontiguousarray(np.asarray(c_image, np.float32)[sl].T).astype(bf)
        m["xt_a"] = np.ascontiguousarray(np.asarray(c_audio, np.float32)[sl].T).astype(bf)
        in_maps.append(m)
    nc = _get_program(NBLK_HW)
    res = bass_utils.run_bass_kernel_spmd(nc, in_maps, core_ids=list(range(NCORES)),
                                          trace=_trace)
    out = np.concatenate([np.asarray(r["out_t"]).astype(np.float32).T for r in res.results], axis=0)
    if _trace:
        kernel.last_results = res
    return out
